# revision 66
# baseline (speedup 1.0000x reference)
"""Distributed Trainium2 kernel for EnhancedSelfAttention (causal attention
with additive ALiBi |i-j| bias) on 8 NeuronCores.

Math: for keys j<=i the bias slope*(i-j) reduces (after the per-row constant
cancels in softmax) to weights w_j = exp(-m*j).  Split w_j = blk * inb with
blk = exp(-m*128*(j//128)) folded into V's key-block rows (and the appended
den column), and inb = exp(-m*(j%128)) folded into the exp as a per-partition
ACT bias.  P' = exp(S/8192 - m*p) then feeds a single PV matmul per block
pair plus a divide; blk scaling makes far blocks underflow to exactly the
truncation the per-slot budgets assume.

Precision: Q/K are produced from fp8(e4m3) x and 32x-prescaled fp8 weights
via DoubleRow matmuls (2 contraction chunks per instruction), attention
P'/V run in fp8 with DoubleRow PV for the far-context slots (0,1) on query
chunks >= 1, and in fp16 for query chunk 0 / fast-decay slots where few keys
get no averaging dilution.  Projection stays fp16.  Measured end-to-end
max-rel error ~1.2e-2 vs the 2e-2 gate.

Sharding: 8 cores = 2 batches x 4 head groups; heads sorted by slope into 4
slots of uniform budget KBUD=(16,4,1,1) key blocks; partial projection
outputs summed on the host.
"""

import sys
import types

import numpy as np
import ml_dtypes

import concourse.bass as bass
import concourse.mybir as mybir
import concourse.tile as tile
from concourse import bacc
from concourse.bass_utils import run_bass_kernel_spmd


def _ensure_axon_hooks():
    try:
        import antenv.axon_hooks  # noqa: F401
    except Exception:
        try:
            import antenv
            mod = types.ModuleType("antenv.axon_hooks")
            mod.get_axon_ntff_profile_hook = lambda: None
            mod.set_axon_ntff_profile_hook = lambda h: None
            sys.modules["antenv.axon_hooks"] = mod
            antenv.axon_hooks = mod
        except Exception:
            pass


_ensure_axon_hooks()

F32 = mybir.dt.float32
F16 = mybir.dt.float16
F8 = mybir.dt.float8e4
DR = mybir.MatmulPerfMode.DoubleRow
ExpF = mybir.ActivationFunctionType.Exp
E4NP = ml_dtypes.float8_e4m3

B, T, C = 2, 2048, 1024
NH, D = 16, 64
P = 128
NT = T // P            # 16 key/t blocks
KC = C // P            # 8 contraction chunks
QCH = 4                # q chunks of 512
KBUD = (16, 4, 1, 1)   # per-slot key-block budgets
N_CORES = 8

# slot s of group g runs head SLOT_HEADS[s][g] (sorted by ALiBi slope so a
# slot's budget is the max need of its heads).
SLOT_HEADS = [[15 - g for g in range(4)], [11 - g for g in range(4)],
              [7 - g for g in range(4)], [3 - g for g in range(4)]]

TRACE = False

_CACHE = {}


def _slopes():
    i = np.arange(1, NH + 1, dtype=np.float64)
    return 1.0 / np.power(2.0, 8.0 * i / NH)


def _build_program():
    nc = bacc.Bacc("TRN2", target_bir_lowering=False, debug=False,
                   num_devices=N_CORES)

    # All inputs are host-pre-arranged to [128, ...] partition-major layouts
    # so every DMA moves long contiguous runs per partition row.
    xt8_d = nc.dram_tensor("xt8", [P, QCH * KC * 512], F8,
                           kind="ExternalInput").ap()
    xtf_d = nc.dram_tensor("xtf", [P, KC * 512], F16, kind="ExternalInput").ap()
    wq8_d = nc.dram_tensor("wq8", [P, KC * 4 * D], F8, kind="ExternalInput").ap()
    wk8_d = nc.dram_tensor("wk8", [P, KC * 4 * D], F8, kind="ExternalInput").ap()
    wv_d = nc.dram_tensor("wv", [P, KC * 4 * D], F16, kind="ExternalInput").ap()
    wp_d = nc.dram_tensor("wp", [P, 2 * C], F16, kind="ExternalInput").ap()
    masks_d = nc.dram_tensor("masks", [P, 4 * 512], F8, kind="ExternalInput").ap()
    bias_d = nc.dram_tensor("bias", [P, 4], F32, kind="ExternalInput").ap()
    vsc_d = nc.dram_tensor("vsc", [P, NT * 4], F32, kind="ExternalInput").ap()
    y_d = nc.dram_tensor("y", [T, C], F16, kind="ExternalOutput").ap()

    with tile.TileContext(nc) as tc:
        with (
            nc.allow_low_precision(reason="fp8/fp16 matmul operands by design"),
            tc.tile_pool(name="const", bufs=1) as const,
            tc.tile_pool(name="psA", bufs=2, space="PSUM") as psA,
            tc.tile_pool(name="psO", bufs=3, space="PSUM") as psO,
            tc.tile_pool(name="psR", bufs=1, space="PSUM") as psR,
            tc.tile_pool(name="pp", bufs=4) as pp,
            tc.tile_pool(name="rr", bufs=3) as rr,
            tc.tile_pool(name="rbp", bufs=2) as rbp,
            tc.tile_pool(name="yp", bufs=4) as yp,
        ):
            # ---- persistent SBUF loads (order = first use)
            wq8_sb = const.tile([P, KC, 4 * D], F8, tag="wq8")
            wk8_sb = const.tile([P, KC, 4 * D], F8, tag="wk8")
            wq8_r = wq8_d.rearrange("p (k n) -> p k n", k=KC)
            nc.sync.dma_start(wq8_sb[:, 0:2], wq8_r[:, 0:2])
            nc.sync.dma_start(wq8_sb[:, 2:KC], wq8_r[:, 2:KC])
            nc.sync.dma_start(wk8_sb[:], wk8_d.rearrange("p (k n) -> p k n", k=KC))
            xt8_sb = const.tile([P, QCH, KC, 512], F8, tag="xt8")
            xt8_r = xt8_d.rearrange("p (n k t) -> p n k t", n=QCH, k=KC)
            nc.sync.dma_start(xt8_sb[:, 0], xt8_r[:, 0])
            wv_sb = const.tile([P, KC, 4 * D], F16, tag="wv")
            nc.sync.dma_start(wv_sb[:], wv_d.rearrange("p (k n) -> p k n", k=KC))
            xtf_sb = const.tile([P, KC, 512], F16, tag="xtf")
            nc.sync.dma_start(xtf_sb[:], xtf_d.rearrange("p (k t) -> p k t", k=KC))
            masks_sb = const.tile([P, 4 * 512], F8, tag="masks")
            nc.sync.dma_start(masks_sb[:], masks_d[:])
            bias_sb = const.tile([P, 4], F32, tag="bias")
            nc.sync.dma_start(bias_sb[:], bias_d[:])
            vsc_sb = const.tile([P, NT, 4], F32, tag="vsc")
            nc.sync.dma_start(vsc_sb[:], vsc_d.rearrange("p (n s) -> p n s", s=4))
            for n in range(1, QCH):
                nc.sync.dma_start(xt8_sb[:, n], xt8_r[:, n])
            wp_sb = const.tile([P, 2, C], F16, tag="wp")
            nc.sync.dma_start(wp_sb[:], wp_d.rearrange("p (j c) -> p j c", j=2))

            ones_sb = const.tile([1, 512], F16, tag="ones")
            nc.any.memset(ones_sb[:], 1.0)
            # warm the ACT exp table + the PE HAM clock gate during DMA wait
            warm_sb = const.tile([1, D], F16, tag="warm")
            nc.scalar.activation(warm_sb[:], ones_sb[0:1, 0:D], ExpF)
            wps = psA.tile([P, 1024], F32, tag="mm", name="ps_warm")
            for i in range(24):
                nc.tensor.matmul(wps[:, 0:256], wq8_sb[:, 0, 0:P],
                                 wq8_sb[:, 0, :], start=True, stop=True)

            qt_sb = [const.tile([P, T], F16, tag=f"qt{m}", name=f"qt{m}")
                     for m in range(2)]
            kt0_sb = const.tile([P, T], F16, tag="kt0")
            kt1_sb = const.tile([P, P], F16, tag="kt1")
            vv8_sb = const.tile([P, NT, 2, 72], F8, tag="vv8")
            vv16_sb = const.tile([P, 4, 4, 66], F16, tag="vv16")
            ot_sb = const.tile([P, 2, T], F16, tag="ot")

            # den columns = per-block factor exp(-m*128*kt)
            nc.vector.tensor_copy(vv8_sb[:, :, :, 64], vsc_sb[:, :, 0:2])
            nc.vector.tensor_copy(vv16_sb[:, :, :, 64], vsc_sb[:, 0:4, :])

            # ---- phase 1: Q^T/K^T via fp8 DoubleRow (2 contraction chunks
            # per matmul), V via fp16 (blocks 0-3 from fp16 x, 4-15 from fp8
            # x stationary with fp16 wv moving, slot0 columns only).
            def emit_qkt(which, m, nch, width=512):
                w_sb, dst = ((wq8_sb, qt_sb[m]),
                             (wk8_sb, kt0_sb if m == 0 else kt1_sb))[which]
                ps = psA.tile([P, 1024], F32, tag="mm", name="ps_qkt")
                for kp in range(KC // 2):
                    nc.tensor.matmul(
                        ps[:, 0:width],
                        w_sb[:, 2 * kp:2 * kp + 2, m * P:(m + 1) * P],
                        xt8_sb[:, nch, 2 * kp:2 * kp + 2, 0:width],
                        start=(kp == 0), stop=(kp == KC // 2 - 1),
                        perf_mode=DR)
                if which == 1 and m == 1:
                    nc.scalar.copy(dst[:, 0:width], ps[:, 0:width])
                elif (which + m + nch) % 2 == 0:
                    nc.scalar.copy(
                        dst[:, nch * 512:nch * 512 + width], ps[:, 0:width])
                else:
                    nc.vector.tensor_copy(
                        dst[:, nch * 512:nch * 512 + width], ps[:, 0:width])

            def emit_v_near(mt):
                nlive = sum(1 for s in range(4) if mt < KBUD[s])
                psv = psA.tile([P, 1024], F32, tag="mm", name="ps_vn")
                for k in range(KC):
                    nc.tensor.matmul(
                        psv[:, 0:nlive * D],
                        xtf_sb[:, k, mt * P:(mt + 1) * P],
                        wv_sb[:, k, 0:nlive * D],
                        start=(k == 0), stop=(k == KC - 1))
                for s in range(nlive):
                    nc.scalar.mul(
                        vv16_sb[:, mt, s, 0:D], psv[:, s * D:(s + 1) * D],
                        vsc_sb[:, mt, s:s + 1])
                for s in range(min(nlive, 2)):
                    nc.vector.tensor_scalar_mul(
                        vv8_sb[:, mt, s, 0:D], psv[:, s * D:(s + 1) * D],
                        vsc_sb[:, mt, s:s + 1])

            def emit_v_far(mt):
                psv = psA.tile([P, 1024], F32, tag="mm", name="ps_vf")
                for k in range(KC):
                    nc.tensor.matmul(
                        psv[:, 0:D],
                        xt8_sb[:, mt // 4, k, (mt % 4) * P:(mt % 4 + 1) * P],
                        wv_sb[:, k, 0:D],
                        start=(k == 0), stop=(k == KC - 1))
                nc.vector.tensor_scalar_mul(
                    vv8_sb[:, mt, 0, 0:D], psv[:, 0:D], vsc_sb[:, mt, 0:1])

            # Upfront: only what query-chunk 0 needs.  Everything else (QT/KT
            # for later chunks, far V blocks) is injected into earlier chunks'
            # strips as PE filler, hiding it under the ACT-bound exp stream.
            emit_qkt(0, 0, 0)
            emit_qkt(0, 1, 0)
            emit_qkt(1, 0, 0)
            emit_qkt(1, 1, 0, width=P)
            for mt in range(4):
                emit_v_near(mt)
            for mt in range(4, 8):
                emit_v_far(mt)
            fillers = {
                0: [lambda w=w, m=m: emit_qkt(w, m, 1)
                    for (w, m) in ((0, 0), (0, 1), (1, 0))],
                1: [lambda w=w, m=m: emit_qkt(w, m, 2)
                    for (w, m) in ((0, 0), (0, 1), (1, 0))]
                   + [lambda t=t: emit_v_far(t) for t in range(8, 12)],
                2: [lambda w=w, m=m: emit_qkt(w, m, 3)
                    for (w, m) in ((0, 0), (0, 1), (1, 0))]
                   + [lambda t=t: emit_v_far(t) for t in range(12, 16)],
            }

            # ---- phase 2: attention (qc-major so projection of finished
            # query chunks overlaps later chunks) with the classic pipeline:
            # S(i) matmuls, exp(i), PV(i-1); paired divides; proj interleave.
            strips = []
            for qc in range(QCH):
                for s in (2, 3, 0, 1):  # small slots first: their divides
                    kmax = min(KBUD[s], 4 * qc + 4)   # overlap the big slots
                    for g in range((kmax + 1) // 2):
                        kts = [kt for kt in (2 * g, 2 * g + 1) if kt < kmax]
                        strips.append((s, qc, g, kts, kmax))

            opsums = {}
            pendings = []  # PV runs 2 strips behind its exp
            actions = []   # deferred (emit_at, fn) actions
            proj_done = [False] * NT

            def emit_pv(strip, pst):
                s, qc, g, kts, kmax = strip
                fp8 = (s <= 1 and qc >= 1)
                op = opsums[(s, qc)]
                if fp8:
                    nc.tensor.matmul(
                        op[:], vv8_sb[:, 2 * g:2 * g + 2, s, 0:65],
                        pst[:, 0:2, :],
                        start=(g == 0), stop=(2 * g + 2 >= kmax),
                        perf_mode=DR)
                else:
                    for d_, kt in enumerate(kts):
                        nc.tensor.matmul(
                            op[:], vv16_sb[:, kt, s, 0:65], pst[:, d_, :],
                            start=(kt == 0), stop=(kt == kmax - 1))

            dhs = {}

            def emit_dh(s_, qc):
                # stage 1: pull the den row to SBUF right after the PV stops,
                # so the later broadcast matmul never waits on the DVE.
                dh = rr.tile([1, 512], F16, tag="dh", name="dh")
                nc.vector.tensor_copy(dh[:], opsums[(s_, qc)][64:65, :])
                dhs[(s_, qc)] = dh

            def emit_divide(sa, qc):
                for s_ in (sa, sa + 1):
                    op = opsums.pop((s_, qc))
                    dh = dhs.pop((s_, qc))
                    rb = psR.tile([D, 512], F32, tag="rb", name="rb")
                    nc.tensor.matmul(rb[:], ones_sb[0:1, 0:D], dh[:],
                                     start=True, stop=True)
                    rbs = rbp.tile([D, 512], F32, tag="rbs", name="rbs")
                    nc.vector.reciprocal_approx_fast(rbs[:], rb[:])
                    base = (s_ % 2) * D
                    nc.vector.tensor_mul(
                        ot_sb[base:base + D, sa // 2, qc * 512:(qc + 1) * 512],
                        op[0:64, :], rbs[:])

            def emit_proj(mt):
                ps = psA.tile([P, 1024], F32, tag="mm", name="ps_proj")
                for nch2 in range(2):
                    for j in range(2):
                        nc.tensor.matmul(
                            ps[:, nch2 * 512:(nch2 + 1) * 512],
                            ot_sb[:, j, mt * P:(mt + 1) * P],
                            wp_sb[:, j, nch2 * 512:(nch2 + 1) * 512],
                            start=(j == 0), stop=(j == 1))
                yt = yp.tile([P, 1024], F16, tag="y", name="yt")
                # evict halves on BOTH engines in parallel: frees the shared
                # psA buffer ~2x sooner, so the next strip's S matmuls don't
                # WAR-wait on the projection psum.
                nc.scalar.copy(yt[:, 0:512], ps[:, 0:512])
                nc.vector.tensor_copy(yt[:, 512:1024], ps[:, 512:1024])
                nc.sync.dma_start(y_d[mt * P:(mt + 1) * P, :], yt[:])
                proj_done[mt] = True

            for i, strip in enumerate(strips):
                s, qc, g, kts, kmax = strip
                fp8 = (s <= 1 and qc >= 1)
                if (s, qc) not in opsums:
                    opsums[(s, qc)] = psO.tile([65, 512], F32, tag="o",
                                               name="opsum")
                if s < 2:
                    qt_t, kt_t = qt_sb[0], kt0_sb
                else:
                    qt_t, kt_t = qt_sb[1], kt1_sb
                base = (s % 2) * D
                w = len(kts)
                sps = psA.tile([P, 1024], F32, tag="mm", name="sps")
                for d_, kt in enumerate(kts):
                    nc.tensor.matmul(
                        sps[:, d_ * 512:(d_ + 1) * 512],
                        kt_t[base:base + D, kt * P:(kt + 1) * P],
                        qt_t[base:base + D, qc * 512:(qc + 1) * 512],
                        start=True, stop=True)
                pst = pp.tile([P, 2, 512], F8 if fp8 else F16,
                              tag="p8" if fp8 else "p16", name="pst")
                nc.scalar.activation(pst[:, 0:w, :], sps[:, 0:512 * w].rearrange(
                    "p (w n) -> p w n", n=512),
                    ExpF, bias=bias_sb[:, s:s + 1], scale=1.0 / 8192.0)
                if g == 2 * qc:
                    nc.vector.tensor_mul(
                        pst[:, 0:w, :],
                        pst[:, 0:w, :],
                        masks_sb[:, 0:512 * w].rearrange("p (w n) -> p w n", n=512))
                elif g == 2 * qc + 1:
                    nc.vector.tensor_mul(
                        pst[:, 0:w, :],
                        pst[:, 0:w, :],
                        masks_sb[:, 1024:1024 + 512 * w].rearrange(
                            "p (w n) -> p w n", n=512))
                while actions and actions[0][0] <= i:
                    actions.pop(0)[1]()
                fl = fillers.get(qc)
                if fl and (qc != 0 or i % 2 == 0):
                    fl.pop(0)()
                if len(pendings) >= 2:
                    pstrip, ppst = pendings.pop(0)
                    emit_pv(pstrip, ppst)
                    ps_, qc_ = pstrip[0], pstrip[1]
                    nxt = pendings[0][0] if pendings else None
                    if nxt is None or (nxt[0], nxt[1]) != (ps_, qc_):
                        emit_dh(ps_, qc_)
                        if ps_ % 2 == 1:
                            actions.append((i + 2, (lambda a=ps_ - 1, b=qc_:
                                                    emit_divide(a, b))))
                            if ps_ == 1:  # s1 closes the chunk: queue projs
                                mts = [4 * qc_ + t for t in range(4)]
                                for off, mt in enumerate(mts):
                                    actions.append((i + 3 + 2 * off,
                                                    (lambda m=mt: emit_proj(m))))
                            actions.sort(key=lambda a: a[0])
                pendings.append((strip, pst))
            for pstrip, ppst in pendings:
                emit_pv(pstrip, ppst)
                ps_, qc_ = pstrip[0], pstrip[1]
                if (ps_, qc_) not in dhs and (ps_, qc_) in opsums:
                    emit_dh(ps_, qc_)
            for _, fn in sorted(actions, key=lambda a: a[0]):
                fn()
            for sa in (0, 2):
                if (sa, 3) in opsums:
                    emit_divide(sa, 3)
            for mt in range(NT):
                if not proj_done[mt]:
                    emit_proj(mt)

    nc.compile()
    return nc


def _host_prep(x, w_qkv, w_proj):
    slopes = _slopes()
    in_maps = []
    xt_by_b = [np.ascontiguousarray(x[b].T) for b in range(B)]

    rr_ = np.arange(P)[:, None]
    cc = np.arange(512)[None, :]
    masks = np.concatenate(
        [(rr_ <= cc - P * d).astype(E4NP) for d in range(4)], axis=1)


    def chunk_major(a):
        # [C, N] -> [P, KC*N]: partition-major with contraction chunks inline
        n = a.shape[1]
        return np.ascontiguousarray(
            a.reshape(KC, P, n).transpose(1, 0, 2).reshape(P, KC * n))

    xt8_by_b, xtf_by_b = [], []
    for b in range(B):
        xt = xt_by_b[b]
        x8 = np.clip(xt, -240, 240).astype(E4NP)
        # [C, T] -> [P, QCH, KC, 512] (query-chunk major)
        xt8_by_b.append(np.ascontiguousarray(
            x8.reshape(KC, P, QCH, 512).transpose(1, 2, 0, 3).reshape(P, -1)))
        xtf_by_b.append(chunk_major(xt[:, 0:512].astype(np.float16)))

    group_data = []
    for g in range(4):
        H = [SLOT_HEADS[s][g] for s in range(4)]
        cols = np.concatenate([np.arange(h * D, (h + 1) * D) for h in H])
        wq8 = chunk_major((32.0 * w_qkv[:, cols]).astype(E4NP))
        wk8 = chunk_major((32.0 * w_qkv[:, C + cols]).astype(E4NP))
        wv = chunk_major(w_qkv[:, 2 * C + cols].astype(np.float16))
        wp = np.ascontiguousarray(
            w_proj[cols, :].astype(np.float16).reshape(2, P, C)
            .transpose(1, 0, 2).reshape(P, 2 * C))
        bias = np.stack(
            [-slopes[h] * np.arange(P, dtype=np.float64) for h in H],
            axis=1).astype(np.float32)
        vsc = np.broadcast_to(
            np.exp(-np.outer(128.0 * np.arange(NT),
                             np.array([slopes[h] for h in H]))
                   ).astype(np.float32).reshape(1, NT * 4),
            (P, NT * 4)).copy()
        group_data.append((wq8, wk8, wv, wp, bias, vsc))

    for c in range(N_CORES):
        b, g = divmod(c, 4)
        wq8, wk8, wv, wp, bias, vsc = group_data[g]
        in_maps.append({
            "xt8": xt8_by_b[b], "xtf": xtf_by_b[b],
            "wq8": wq8, "wk8": wk8, "wv": wv, "wp": wp,
            "masks": masks, "bias": bias, "vsc": vsc,
        })
    return in_maps


def kernel(x, w_qkv, w_proj):
    if "nc" not in _CACHE:
        _CACHE["nc"] = _build_program()
    nc = _CACHE["nc"]

    in_maps = _host_prep(np.asarray(x, np.float32), np.asarray(w_qkv, np.float32),
                         np.asarray(w_proj, np.float32))
    res = run_bass_kernel_spmd(nc, in_maps, list(range(N_CORES)), trace=TRACE)
    _CACHE["last_result"] = res

    y = np.zeros((B, T, C), dtype=np.float64)
    for c in range(N_CORES):
        b = c // 4
        y[b] += res.results[c]["y"].astype(np.float64)
    return y.astype(np.float32)


# revision 67
# speedup vs baseline: 1.0569x; 1.0569x over previous
"""Distributed Trainium2 kernel for EnhancedSelfAttention (causal attention
with additive ALiBi |i-j| bias) on 8 NeuronCores.

Math: for keys j<=i the bias slope*(i-j) reduces (after the per-row constant
cancels in softmax) to weights w_j = exp(-m*j).  Split w_j = blk * inb with
blk = exp(-m*128*(j//128)) folded into V's key-block rows (and the appended
den column), and inb = exp(-m*(j%128)) folded into the exp as a per-partition
ACT bias.  P' = exp(S/8192 - m*p) then feeds a single PV matmul per block
pair plus a divide; blk scaling makes far blocks underflow to exactly the
truncation the per-slot budgets assume.

Precision: Q/K are produced from fp8(e4m3) x and 32x-prescaled fp8 weights
via DoubleRow matmuls (2 contraction chunks per instruction), attention
P'/V run in fp8 with DoubleRow PV for the far-context slots (0,1) on query
chunks >= 1, and in fp16 for query chunk 0 / fast-decay slots where few keys
get no averaging dilution.  Projection stays fp16.  Measured end-to-end
max-rel error ~1.2e-2 vs the 2e-2 gate.

Sharding: 8 cores = 2 batches x 4 head groups; heads sorted by slope into 4
slots of uniform budget KBUD=(16,4,1,1) key blocks; partial projection
outputs summed on the host.
"""

import sys
import types

import numpy as np
import ml_dtypes

import concourse.bass as bass
import concourse.mybir as mybir
import concourse.tile as tile
from concourse import bacc
from concourse.bass_utils import run_bass_kernel_spmd


def _ensure_axon_hooks():
    try:
        import antenv.axon_hooks  # noqa: F401
    except Exception:
        try:
            import antenv
            mod = types.ModuleType("antenv.axon_hooks")
            mod.get_axon_ntff_profile_hook = lambda: None
            mod.set_axon_ntff_profile_hook = lambda h: None
            sys.modules["antenv.axon_hooks"] = mod
            antenv.axon_hooks = mod
        except Exception:
            pass


_ensure_axon_hooks()

F32 = mybir.dt.float32
F16 = mybir.dt.float16
F8 = mybir.dt.float8e4
DR = mybir.MatmulPerfMode.DoubleRow
ExpF = mybir.ActivationFunctionType.Exp
E4NP = ml_dtypes.float8_e4m3

B, T, C = 2, 2048, 1024
NH, D = 16, 64
P = 128
NT = T // P            # 16 key/t blocks
KC = C // P            # 8 contraction chunks
QCH = 4                # q chunks of 512
KBUD = (16, 4, 1, 1)   # per-slot key-block budgets
N_CORES = 8

# slot s of group g runs head SLOT_HEADS[s][g] (sorted by ALiBi slope so a
# slot's budget is the max need of its heads).
SLOT_HEADS = [[15 - g for g in range(4)], [11 - g for g in range(4)],
              [7 - g for g in range(4)], [3 - g for g in range(4)]]

TRACE = False

_CACHE = {}


def _slopes():
    i = np.arange(1, NH + 1, dtype=np.float64)
    return 1.0 / np.power(2.0, 8.0 * i / NH)


def _build_program():
    nc = bacc.Bacc("TRN2", target_bir_lowering=False, debug=False,
                   num_devices=N_CORES)

    # All inputs are host-pre-arranged to [128, ...] partition-major layouts
    # so every DMA moves long contiguous runs per partition row.
    xt8_d = nc.dram_tensor("xt8", [P, QCH * KC * 512], F8,
                           kind="ExternalInput").ap()
    xtf_d = nc.dram_tensor("xtf", [P, KC * 512], F16, kind="ExternalInput").ap()
    wq8_d = nc.dram_tensor("wq8", [P, KC * 4 * D], F8, kind="ExternalInput").ap()
    wk8_d = nc.dram_tensor("wk8", [P, KC * 4 * D], F8, kind="ExternalInput").ap()
    wv_d = nc.dram_tensor("wv", [P, KC * 4 * D], F16, kind="ExternalInput").ap()
    wp_d = nc.dram_tensor("wp", [P, 2 * C], F16, kind="ExternalInput").ap()
    masks_d = nc.dram_tensor("masks", [P, 4 * 512], F8, kind="ExternalInput").ap()
    bias_d = nc.dram_tensor("bias", [P, 4], F32, kind="ExternalInput").ap()
    vsc_d = nc.dram_tensor("vsc", [P, NT * 4], F32, kind="ExternalInput").ap()
    y_d = nc.dram_tensor("y", [T, C], F16, kind="ExternalOutput").ap()

    with tile.TileContext(nc) as tc:
        with (
            nc.allow_low_precision(reason="fp8/fp16 matmul operands by design"),
            tc.tile_pool(name="const", bufs=1) as const,
            tc.tile_pool(name="psA", bufs=2, space="PSUM") as psA,
            tc.tile_pool(name="psO", bufs=3, space="PSUM") as psO,
            tc.tile_pool(name="psR", bufs=1, space="PSUM") as psR,
            tc.tile_pool(name="pp", bufs=4) as pp,
            tc.tile_pool(name="rr", bufs=3) as rr,
            tc.tile_pool(name="rbp", bufs=2) as rbp,
            tc.tile_pool(name="yp", bufs=4) as yp,
        ):
            # ---- persistent SBUF loads (order = first use)
            wq8_sb = const.tile([P, KC, 4 * D], F8, tag="wq8")
            wk8_sb = const.tile([P, KC, 4 * D], F8, tag="wk8")
            wq8_r = wq8_d.rearrange("p (k n) -> p k n", k=KC)
            nc.sync.dma_start(wq8_sb[:, 0:2], wq8_r[:, 0:2])
            nc.sync.dma_start(wq8_sb[:, 2:KC], wq8_r[:, 2:KC])
            nc.sync.dma_start(wk8_sb[:], wk8_d.rearrange("p (k n) -> p k n", k=KC))
            xt8_sb = const.tile([P, QCH, KC, 512], F8, tag="xt8")
            xt8_r = xt8_d.rearrange("p (n k t) -> p n k t", n=QCH, k=KC)
            nc.sync.dma_start(xt8_sb[:, 0], xt8_r[:, 0])
            wv_sb = const.tile([P, KC, 4 * D], F16, tag="wv")
            nc.sync.dma_start(wv_sb[:], wv_d.rearrange("p (k n) -> p k n", k=KC))
            xtf_sb = const.tile([P, KC, 512], F16, tag="xtf")
            nc.sync.dma_start(xtf_sb[:], xtf_d.rearrange("p (k t) -> p k t", k=KC))
            masks_sb = const.tile([P, 4 * 512], F8, tag="masks")
            nc.sync.dma_start(masks_sb[:], masks_d[:])
            bias_sb = const.tile([P, 4], F32, tag="bias")
            nc.sync.dma_start(bias_sb[:], bias_d[:])
            vsc_sb = const.tile([P, NT, 4], F32, tag="vsc")
            nc.sync.dma_start(vsc_sb[:], vsc_d.rearrange("p (n s) -> p n s", s=4))
            for n in range(1, QCH):
                nc.sync.dma_start(xt8_sb[:, n], xt8_r[:, n])
            wp_sb = const.tile([P, 2, C], F16, tag="wp")
            nc.sync.dma_start(wp_sb[:], wp_d.rearrange("p (j c) -> p j c", j=2))

            ones_sb = const.tile([1, 512], F16, tag="ones")
            nc.any.memset(ones_sb[:], 1.0)
            # warm the ACT exp table + the PE HAM clock gate during DMA wait
            warm_sb = const.tile([1, D], F16, tag="warm")
            nc.scalar.activation(warm_sb[:], ones_sb[0:1, 0:D], ExpF)
            wps = psA.tile([P, 1024], F32, tag="mm", name="ps_warm")
            for i in range(24):
                nc.tensor.matmul(wps[:, 0:256], wq8_sb[:, 0, 0:P],
                                 wq8_sb[:, 0, :], start=True, stop=True)

            qt_sb = [const.tile([P, T], F16, tag=f"qt{m}", name=f"qt{m}")
                     for m in range(2)]
            kt0_sb = const.tile([P, T], F16, tag="kt0")
            kt1_sb = const.tile([P, P], F16, tag="kt1")
            vv8_sb = const.tile([P, NT, 2, 72], F8, tag="vv8")
            vv16_sb = const.tile([P, 4, 4, 66], F16, tag="vv16")
            ot_sb = const.tile([P, 2, T], F16, tag="ot")

            # den columns = per-block factor exp(-m*128*kt)
            nc.vector.tensor_copy(vv8_sb[:, :, :, 64], vsc_sb[:, :, 0:2])
            nc.vector.tensor_copy(vv16_sb[:, :, :, 64], vsc_sb[:, 0:4, :])

            # ---- phase 1: Q^T/K^T via fp8 DoubleRow (2 contraction chunks
            # per matmul), V via fp16 (blocks 0-3 from fp16 x, 4-15 from fp8
            # x stationary with fp16 wv moving, slot0 columns only).
            def emit_qkt(which, m, nch, width=512):
                w_sb, dst = ((wq8_sb, qt_sb[m]),
                             (wk8_sb, kt0_sb if m == 0 else kt1_sb))[which]
                ps = psA.tile([P, 1024], F32, tag="mm", name="ps_qkt")
                for kp in range(KC // 2):
                    nc.tensor.matmul(
                        ps[:, 0:width],
                        w_sb[:, 2 * kp:2 * kp + 2, m * P:(m + 1) * P],
                        xt8_sb[:, nch, 2 * kp:2 * kp + 2, 0:width],
                        start=(kp == 0), stop=(kp == KC // 2 - 1),
                        perf_mode=DR)
                if which == 1 and m == 1:
                    nc.scalar.copy(dst[:, 0:width], ps[:, 0:width])
                elif (which + m + nch) % 2 == 0:
                    nc.scalar.copy(
                        dst[:, nch * 512:nch * 512 + width], ps[:, 0:width])
                else:
                    nc.vector.tensor_copy(
                        dst[:, nch * 512:nch * 512 + width], ps[:, 0:width])

            def emit_v_near(mt):
                nlive = sum(1 for s in range(4) if mt < KBUD[s])
                psv = psA.tile([P, 1024], F32, tag="mm", name="ps_vn")
                for k in range(KC):
                    nc.tensor.matmul(
                        psv[:, 0:nlive * D],
                        xtf_sb[:, k, mt * P:(mt + 1) * P],
                        wv_sb[:, k, 0:nlive * D],
                        start=(k == 0), stop=(k == KC - 1))
                for s in range(nlive):
                    nc.scalar.mul(
                        vv16_sb[:, mt, s, 0:D], psv[:, s * D:(s + 1) * D],
                        vsc_sb[:, mt, s:s + 1])
                for s in range(min(nlive, 2)):
                    nc.vector.tensor_scalar_mul(
                        vv8_sb[:, mt, s, 0:D], psv[:, s * D:(s + 1) * D],
                        vsc_sb[:, mt, s:s + 1])

            def emit_v_far(mt):
                psv = psA.tile([P, 1024], F32, tag="mm", name="ps_vf")
                for k in range(KC):
                    nc.tensor.matmul(
                        psv[:, 0:D],
                        xt8_sb[:, mt // 4, k, (mt % 4) * P:(mt % 4 + 1) * P],
                        wv_sb[:, k, 0:D],
                        start=(k == 0), stop=(k == KC - 1))
                nc.vector.tensor_scalar_mul(
                    vv8_sb[:, mt, 0, 0:D], psv[:, 0:D], vsc_sb[:, mt, 0:1])

            # Upfront: only what query-chunk 0 needs.  Everything else (QT/KT
            # for later chunks, far V blocks) is injected into earlier chunks'
            # strips as PE filler, hiding it under the ACT-bound exp stream.
            emit_qkt(0, 0, 0)
            emit_qkt(0, 1, 0)
            emit_qkt(1, 0, 0)
            emit_qkt(1, 1, 0, width=P)
            for mt in range(4):
                emit_v_near(mt)
            for mt in range(4, 8):
                emit_v_far(mt)
            fillers = {
                0: [lambda w=w, m=m: emit_qkt(w, m, 1)
                    for (w, m) in ((0, 0), (0, 1), (1, 0))],
                1: [lambda w=w, m=m: emit_qkt(w, m, 2)
                    for (w, m) in ((0, 0), (0, 1), (1, 0))]
                   + [lambda t=t: emit_v_far(t) for t in range(8, 12)],
                2: [lambda w=w, m=m: emit_qkt(w, m, 3)
                    for (w, m) in ((0, 0), (0, 1), (1, 0))]
                   + [lambda t=t: emit_v_far(t) for t in range(12, 16)],
            }

            # ---- phase 2: attention (qc-major so projection of finished
            # query chunks overlaps later chunks) with the classic pipeline:
            # S(i) matmuls, exp(i), PV(i-1); paired divides; proj interleave.
            strips = []
            for qc in range(QCH):
                for s in (2, 3, 0, 1):  # small slots first: their divides
                    kmax = min(KBUD[s], 4 * qc + 4)   # overlap the big slots
                    for g in range((kmax + 1) // 2):
                        kts = [kt for kt in (2 * g, 2 * g + 1) if kt < kmax]
                        strips.append((s, qc, g, kts, kmax))

            opsums = {}
            pendings = []  # PV runs 2 strips behind its exp
            actions = []   # deferred (emit_at, fn) actions
            proj_done = [False] * NT

            def emit_pv(strip, pst):
                s, qc, g, kts, kmax = strip
                fp8 = (s <= 1 and qc >= 1)
                op = opsums[(s, qc)]
                if fp8:
                    nc.tensor.matmul(
                        op[:], vv8_sb[:, 2 * g:2 * g + 2, s, 0:65],
                        pst[:, 0:2, :],
                        start=(g == 0), stop=(2 * g + 2 >= kmax),
                        perf_mode=DR)
                else:
                    for d_, kt in enumerate(kts):
                        nc.tensor.matmul(
                            op[:], vv16_sb[:, kt, s, 0:65], pst[:, d_, :],
                            start=(kt == 0), stop=(kt == kmax - 1))

            dhs = {}

            def emit_dh(s_, qc):
                # stage 1: pull the den row to SBUF right after the PV stops,
                # so the later broadcast matmul never waits on the DVE.
                dh = rr.tile([1, 512], F16, tag="dh", name="dh")
                nc.vector.tensor_copy(dh[:], opsums[(s_, qc)][64:65, :])
                dhs[(s_, qc)] = dh

            def emit_divide(sa, qc):
                for s_ in (sa, sa + 1):
                    op = opsums.pop((s_, qc))
                    dh = dhs.pop((s_, qc))
                    rb = psR.tile([D, 512], F32, tag="rb", name="rb")
                    nc.tensor.matmul(rb[:], ones_sb[0:1, 0:D], dh[:],
                                     start=True, stop=True)
                    rbs = rbp.tile([D, 512], F32, tag="rbs", name="rbs")
                    nc.vector.reciprocal_approx_fast(rbs[:], rb[:])
                    base = (s_ % 2) * D
                    nc.vector.tensor_mul(
                        ot_sb[base:base + D, sa // 2, qc * 512:(qc + 1) * 512],
                        op[0:64, :], rbs[:])

            def emit_proj(mt):
                ps = psA.tile([P, 1024], F32, tag="mm", name="ps_proj")
                for nch2 in range(2):
                    for j in range(2):
                        nc.tensor.matmul(
                            ps[:, nch2 * 512:(nch2 + 1) * 512],
                            ot_sb[:, j, mt * P:(mt + 1) * P],
                            wp_sb[:, j, nch2 * 512:(nch2 + 1) * 512],
                            start=(j == 0), stop=(j == 1))
                yt = yp.tile([P, 1024], F16, tag="y", name="yt")
                if mt < 8:
                    nc.scalar.copy(yt[:], ps[:])
                else:   # late window is ACT-bound: keep exp latency low
                    nc.vector.tensor_copy(yt[:], ps[:])
                nc.sync.dma_start(y_d[mt * P:(mt + 1) * P, :], yt[:])
                proj_done[mt] = True

            for i, strip in enumerate(strips):
                s, qc, g, kts, kmax = strip
                fp8 = (s <= 1 and qc >= 1)
                if (s, qc) not in opsums:
                    opsums[(s, qc)] = psO.tile([65, 512], F32, tag="o",
                                               name="opsum")
                if s < 2:
                    qt_t, kt_t = qt_sb[0], kt0_sb
                else:
                    qt_t, kt_t = qt_sb[1], kt1_sb
                base = (s % 2) * D
                w = len(kts)
                sps = psA.tile([P, 1024], F32, tag="mm", name="sps")
                for d_, kt in enumerate(kts):
                    nc.tensor.matmul(
                        sps[:, d_ * 512:(d_ + 1) * 512],
                        kt_t[base:base + D, kt * P:(kt + 1) * P],
                        qt_t[base:base + D, qc * 512:(qc + 1) * 512],
                        start=True, stop=True)
                pst = pp.tile([P, 2, 512], F8 if fp8 else F16,
                              tag="p8" if fp8 else "p16", name="pst")
                nc.scalar.activation(pst[:, 0:w, :], sps[:, 0:512 * w].rearrange(
                    "p (w n) -> p w n", n=512),
                    ExpF, bias=bias_sb[:, s:s + 1], scale=1.0 / 8192.0)
                if g == 2 * qc:
                    nc.vector.tensor_mul(
                        pst[:, 0:w, :],
                        pst[:, 0:w, :],
                        masks_sb[:, 0:512 * w].rearrange("p (w n) -> p w n", n=512))
                elif g == 2 * qc + 1:
                    nc.vector.tensor_mul(
                        pst[:, 0:w, :],
                        pst[:, 0:w, :],
                        masks_sb[:, 1024:1024 + 512 * w].rearrange(
                            "p (w n) -> p w n", n=512))
                while actions and actions[0][0] <= i:
                    actions.pop(0)[1]()
                fl = fillers.get(qc)
                if fl and (qc != 0 or i % 2 == 0):
                    fl.pop(0)()
                if len(pendings) >= 2:
                    pstrip, ppst = pendings.pop(0)
                    emit_pv(pstrip, ppst)
                    ps_, qc_ = pstrip[0], pstrip[1]
                    nxt = pendings[0][0] if pendings else None
                    if nxt is None or (nxt[0], nxt[1]) != (ps_, qc_):
                        emit_dh(ps_, qc_)
                        if ps_ % 2 == 1:
                            actions.append((i + 2, (lambda a=ps_ - 1, b=qc_:
                                                    emit_divide(a, b))))
                            if ps_ == 1:  # s1 closes the chunk: queue projs
                                mts = [4 * qc_ + t for t in range(4)]
                                for off, mt in enumerate(mts):
                                    actions.append((i + 3 + 2 * off,
                                                    (lambda m=mt: emit_proj(m))))
                            actions.sort(key=lambda a: a[0])
                pendings.append((strip, pst))
            for pstrip, ppst in pendings:
                emit_pv(pstrip, ppst)
                ps_, qc_ = pstrip[0], pstrip[1]
                if (ps_, qc_) not in dhs and (ps_, qc_) in opsums:
                    emit_dh(ps_, qc_)
            for _, fn in sorted(actions, key=lambda a: a[0]):
                fn()
            for sa in (0, 2):
                if (sa, 3) in opsums:
                    emit_divide(sa, 3)
            for mt in range(NT):
                if not proj_done[mt]:
                    emit_proj(mt)

    nc.compile()
    return nc


def _host_prep(x, w_qkv, w_proj):
    slopes = _slopes()
    in_maps = []
    xt_by_b = [np.ascontiguousarray(x[b].T) for b in range(B)]

    rr_ = np.arange(P)[:, None]
    cc = np.arange(512)[None, :]
    masks = np.concatenate(
        [(rr_ <= cc - P * d).astype(E4NP) for d in range(4)], axis=1)


    def chunk_major(a):
        # [C, N] -> [P, KC*N]: partition-major with contraction chunks inline
        n = a.shape[1]
        return np.ascontiguousarray(
            a.reshape(KC, P, n).transpose(1, 0, 2).reshape(P, KC * n))

    xt8_by_b, xtf_by_b = [], []
    for b in range(B):
        xt = xt_by_b[b]
        x8 = np.clip(xt, -240, 240).astype(E4NP)
        # [C, T] -> [P, QCH, KC, 512] (query-chunk major)
        xt8_by_b.append(np.ascontiguousarray(
            x8.reshape(KC, P, QCH, 512).transpose(1, 2, 0, 3).reshape(P, -1)))
        xtf_by_b.append(chunk_major(xt[:, 0:512].astype(np.float16)))

    group_data = []
    for g in range(4):
        H = [SLOT_HEADS[s][g] for s in range(4)]
        cols = np.concatenate([np.arange(h * D, (h + 1) * D) for h in H])
        wq8 = chunk_major((32.0 * w_qkv[:, cols]).astype(E4NP))
        wk8 = chunk_major((32.0 * w_qkv[:, C + cols]).astype(E4NP))
        wv = chunk_major(w_qkv[:, 2 * C + cols].astype(np.float16))
        wp = np.ascontiguousarray(
            w_proj[cols, :].astype(np.float16).reshape(2, P, C)
            .transpose(1, 0, 2).reshape(P, 2 * C))
        bias = np.stack(
            [-slopes[h] * np.arange(P, dtype=np.float64) for h in H],
            axis=1).astype(np.float32)
        vsc = np.broadcast_to(
            np.exp(-np.outer(128.0 * np.arange(NT),
                             np.array([slopes[h] for h in H]))
                   ).astype(np.float32).reshape(1, NT * 4),
            (P, NT * 4)).copy()
        group_data.append((wq8, wk8, wv, wp, bias, vsc))

    for c in range(N_CORES):
        b, g = divmod(c, 4)
        wq8, wk8, wv, wp, bias, vsc = group_data[g]
        in_maps.append({
            "xt8": xt8_by_b[b], "xtf": xtf_by_b[b],
            "wq8": wq8, "wk8": wk8, "wv": wv, "wp": wp,
            "masks": masks, "bias": bias, "vsc": vsc,
        })
    return in_maps


def kernel(x, w_qkv, w_proj):
    if "nc" not in _CACHE:
        _CACHE["nc"] = _build_program()
    nc = _CACHE["nc"]

    in_maps = _host_prep(np.asarray(x, np.float32), np.asarray(w_qkv, np.float32),
                         np.asarray(w_proj, np.float32))
    res = run_bass_kernel_spmd(nc, in_maps, list(range(N_CORES)), trace=TRACE)
    _CACHE["last_result"] = res

    y = np.zeros((B, T, C), dtype=np.float64)
    for c in range(N_CORES):
        b = c // 4
        y[b] += res.results[c]["y"].astype(np.float64)
    return y.astype(np.float32)
